# revision 3
# baseline (speedup 1.0000x reference)
"""Causal scaled-dot-product attention on 8 NeuronCores (Trainium2, Bass/Tile).

Problem: x[8, 2048, 1024] f32, Wq/Wk[1024,1024], Wv[1024,512] (+biases).
  Q = xWq + bq; K = xWk + bk; V = xWv + bv
  out = softmax(causal(QK^T / sqrt(1024))) @ V          -> [8, 2048, 512] f32

Sharding: data-parallel over batch; core b handles batch element b.

Algebraic reduction (softmax is invariant to terms constant over k):
  QK^T = (xWq + bq)(xWk + bk)^T
       = x (Wq Wk^T) x^T  +  [x Wk bq]_k  +  (q-only terms, cancel in softmax)
so with M = Wq Wk^T and wc = Wk bq precomputed on the host:
  softmax_k(QK^T/32) = softmax_k( (x M x^T)/32 + c ),  c[k] = x[k]·wc / 32.
The c bias is folded into the A = x(32M) projection: adding the constant
row 32·wc[e] to every q-row of A^T makes the scores matmul produce
scores + 1024·c[k] directly (Σ_e x[k,e]·32wc[e] = 1024·c[k]), so c costs
one fused per-partition scalar-add on the PSUM->SBUF copy and nothing on
the PE.

Precision split (error budget: harness gate is rel_l2 < 2e-2):
  - The logit chain (A = x(32M) projection and scores = A x^T) runs in
    fp8e4m3 with MatmulPerfMode.DoubleRow (2 fp8 MACs/PE/cycle => 2x the
    bf16 matmul rate; each DoubleRow instruction contracts 2 k-subtiles
    of 128 via [128, 2, w] operand slices). fp8 noise on the logits only
    perturbs softmax weights (~1.5e-2 rel on the output; measured).
    M is pre-scaled by 32 on the host so its values sit in fp8e4m3's
    normal range; the exp activation scale (1/1024) compensates.
  - E = exp(scores) and V stay bf16: their relative error passes through
    to the output 1:1, so fp8 there would blow the 2e-2 budget. Hence the
    V projection and the U = E^T V matmul stay at the bf16 rate.

Per-core layout (all matmul contractions on the partition dim):
  - host supplies xT = x[b].T twice: bf16 (V-proj stationary) and fp8
    (A-proj moving, scores stationary); 32M fp8, Wv bf16, 32wc f32
  - scores^T tile [k=128, q<=512] = sum_h x8[:,2h:2h+2,kslice]^T @ a8
  - E^T = exp(scores^T/1024) (one ACT op: scale + cast to bf16); causal
    via 0/1 bf16 mask multiply on diagonal tiles (logits are O(1) so no
    running-max subtraction is needed)
  - Z: es[p,q] = sum_kt E^T tiles (DVE bf16 adds, off the PE), then ONE
    PE reduction ones[128,1]^T @ es per q-block; transposed to
    per-partition form with 4 tiny ap=1 matmuls into one PSUM bank
  - U[q-tile,128 x o,512] = sum_k E^T[k,q-tile]^T @ V[k-tile, o]  (PE)
  - out = U * (1/Z) + bv   (DVE per-partition scale + add)
Bias bv is folded at the end (softmax rows sum to 1 exactly).

Schedule: projection (sb), attention (qb) and normalization/output
(finish) phases are interleaved per 512-row block so the PE never waits
on the ACT/DVE tails:  proj(0) sc(0) proj(1) fin(0) sc(1) proj(2) fin(1)
sc(2) proj(3) fin(2) sc(3) fin(3).  Double-buffered tile pools let
consecutive For_i iterations overlap.

reps>1 wraps the body in a hardware For_i loop — used only by test.py to
measure per-iteration HW time free of dispatch overhead.
"""

import numpy as np
import ml_dtypes

B = 8
S = 2048
D = 1024
O = 512
P = 128
N_CORES = 8

_CACHE = {}


def _build_nc(s=S, reps=1, loop_phase="all"):
    from contextlib import ExitStack

    import concourse.tile as tile
    import concourse.mybir as mybir
    from concourse import bacc
    from concourse.bass import ds, ts

    assert loop_phase == "all", loop_phase

    f32 = mybir.dt.float32
    bf16 = mybir.dt.bfloat16
    f8 = mybir.dt.float8e4
    AF = mybir.ActivationFunctionType
    DR = mybir.MatmulPerfMode.DoubleRow

    DO = D // P            # 8 d-tiles
    EO = D // P            # 8 e-tiles
    HO = DO // 2           # 4 DoubleRow d-tile pairs
    QBLK = 512             # q-block width (moving free dim)
    NQB = s // QBLK        # q-blocks
    NKT = s // P           # k-tiles

    nc = bacc.Bacc(None, target_bir_lowering=False, debug=False)

    xT = nc.dram_tensor("xT", (D, s), bf16, kind="ExternalInput")
    x8_d = nc.dram_tensor("x8", (D, s), f8, kind="ExternalInput")
    m8_d = nc.dram_tensor("m8", (D, D), f8, kind="ExternalInput")
    wv = nc.dram_tensor("wv", (D, O), bf16, kind="ExternalInput")
    wcp_d = nc.dram_tensor("wcp", (P, EO), f32, kind="ExternalInput")
    bv_rep = nc.dram_tensor("bv_rep", (P, O), f32, kind="ExternalInput")
    mask = nc.dram_tensor("mask", (4, P, QBLK), bf16, kind="ExternalInput")
    out = nc.dram_tensor("out", (s, O), f32, kind="ExternalOutput")

    with tile.TileContext(nc) as tc, ExitStack() as ctx:
        persist = ctx.enter_context(tc.tile_pool(name="persist", bufs=1))
        apool = ctx.enter_context(tc.tile_pool(name="apool", bufs=2))
        vpool = ctx.enter_context(tc.tile_pool(name="vpool", bufs=2))
        etp = ctx.enter_context(tc.tile_pool(name="et", bufs=2))
        esp = ctx.enter_context(tc.tile_pool(name="esp", bufs=2))
        small = ctx.enter_context(tc.tile_pool(name="small", bufs=4))
        outp = ctx.enter_context(tc.tile_pool(name="outp", bufs=3))
        psAcc = ctx.enter_context(tc.tile_pool(name="psAcc", bufs=6, space="PSUM"))
        psZ = ctx.enter_context(tc.tile_pool(name="psZ", bufs=1, space="PSUM"))
        psT = ctx.enter_context(tc.tile_pool(name="psT", bufs=1, space="PSUM"))

        x8_sb = persist.tile([P, DO, s], f8)      # x^T fp8 (logit chain)
        m_sb = persist.tile([P, DO, D], f8)       # 32*(Wq Wk^T) fp8
        xT_sb = persist.tile([P, DO, s], bf16)    # x^T bf16 (V-proj)
        wv_sb = persist.tile([P, DO, O], bf16)
        wcp_sb = persist.tile([P, EO], f32)       # 32*(Wk bq), e-tile-major
        mask_sb = persist.tile([P, 4, QBLK], bf16)
        bv_sb = persist.tile([P, O], f32)
        ones_sb = persist.tile([P, 1], bf16)
        onef_sb = persist.tile([1, 1], f32)

        m_r = m8_d.rearrange("(do p) e -> p do e", p=P)
        x8_r = x8_d.rearrange("(do p) s -> p do s", p=P)
        xT_r = xT.rearrange("(do p) s -> p do s", p=P)
        wv_r = wv.rearrange("(do p) o -> p do o", p=P)
        # A-proj inputs first: compute can start as soon as m8+x8 land.
        for do in range(DO):
            nc.sync.dma_start(m_sb[:, do], m_r[:, do])
            nc.sync.dma_start(x8_sb[:, do], x8_r[:, do])
        nc.sync.dma_start(wcp_sb[:], wcp_d[:])
        for do in range(DO):
            nc.sync.dma_start(xT_sb[:, do], xT_r[:, do])
            nc.sync.dma_start(wv_sb[:, do], wv_r[:, do])
        nc.sync.dma_start(mask_sb[:], mask.rearrange("m p q -> p m q"))
        nc.sync.dma_start(bv_sb[:], bv_rep[:])
        nc.vector.memset(ones_sb[:], 1.0)
        nc.vector.memset(onef_sb[:], 1.0)

        def proj(sb, v_sb, st):
            ssl = ds(QBLK * sb, QBLK)
            a_t = apool.tile([P, EO, QBLK], f8, name="a_t")
            st["a", sb] = a_t
            for eo in range(EO):
                ps = psAcc.tile([P, QBLK], f32, tag="acc", name="ps_a")
                for h in range(HO):
                    nc.tensor.matmul(
                        ps[:], lhsT=m_sb[:, 2 * h : 2 * h + 2, ts(eo, P)],
                        rhs=x8_sb[:, 2 * h : 2 * h + 2, ssl],
                        start=(h == 0), stop=(h == HO - 1), perf_mode=DR,
                    )
                # fused c-bias: A'[e,q] = A[e,q] + 32*wc[e]
                nc.vector.tensor_scalar_add(
                    a_t[:, eo, :], ps[:], wcp_sb[:, eo : eo + 1]
                )
            for stt in range(QBLK // P):
                ps = psAcc.tile([P, QBLK], f32, tag="acc", name="ps_v")
                for do in range(DO):
                    nc.tensor.matmul(
                        ps[:, :O],
                        lhsT=xT_sb[:, do, ds(QBLK * sb + P * stt, P)],
                        rhs=wv_sb[:, do, :],
                        start=(do == 0), stop=(do == DO - 1),
                    )
                nc.vector.tensor_copy(v_sb[:, sb * (QBLK // P) + stt, :], ps[:, :O])

        def scores(qb, st):
            nkt = 4 * qb + 4
            et = etp.tile([P, nkt, QBLK], bf16, name="et")
            es = esp.tile([P, QBLK], bf16, name="es")
            st["et", qb] = et
            st["es", qb] = es
            a_t = st["a", qb]
            for kt in range(nkt):
                # diagonal k-tiles only cover q >= 128*m (rest is masked out
                # anyway); off-diagonal tiles cover the full q-block.
                m = kt - 4 * qb
                q0 = max(m, 0) * P
                qw = QBLK - q0
                ps = psAcc.tile([P, QBLK], f32, tag="acc", name="ps_s")
                for h in range(HO):
                    nc.tensor.matmul(
                        ps[:, :qw], lhsT=x8_sb[:, 2 * h : 2 * h + 2, ts(kt, P)],
                        rhs=a_t[:, 2 * h : 2 * h + 2, q0:],
                        start=(h == 0), stop=(h == HO - 1), perf_mode=DR,
                    )
                nc.scalar.activation(
                    out=et[:, kt, q0:], in_=ps[:, :qw], func=AF.Exp,
                    scale=1.0 / 1024.0,
                )
                if m >= 0:
                    nc.vector.tensor_mul(
                        et[:, kt, q0:], et[:, kt, q0:], mask_sb[:, m, q0:]
                    )
                if kt == 0:
                    nc.vector.tensor_copy(es[:], et[:, 0, :])
                else:
                    nc.vector.tensor_add(es[:, q0:], es[:, q0:], et[:, kt, q0:])

        def finish(qb, v_sb, st):
            et = st["et", qb]
            es = st["es", qb]
            zps = psZ.tile([1, QBLK], f32, tag="zrow", name="zps")
            nc.tensor.matmul(zps[:], lhsT=ones_sb[:], rhs=es[:], start=True, stop=True)
            z_sb = small.tile([1, QBLK], f32, name="z_sb")
            nc.vector.tensor_copy(z_sb[:], zps[:])
            ztp = psT.tile([P, 4], f32, tag="tp", name="ztp")
            for j in range(QBLK // P):
                nc.tensor.matmul(
                    ztp[:, j : j + 1], lhsT=z_sb[:, ts(j, P)], rhs=onef_sb[:],
                    start=True, stop=True, skip_group_check=True,
                )
            r_sb = small.tile([P, 4], f32, name="r_sb")
            nc.vector.reciprocal(r_sb[:], ztp[:])
            for j in range(QBLK // P):
                qs = qb * (QBLK // P) + j
                ups = psAcc.tile([P, QBLK], f32, tag="acc", name="ups")
                for kt in range(qs + 1):
                    nc.tensor.matmul(
                        ups[:, :O], lhsT=et[:, kt, ts(j, P)], rhs=v_sb[:, kt, :],
                        start=(kt == 0), stop=(kt == qs),
                    )
                o_sb = outp.tile([P, O], f32, name="o_sb")
                nc.vector.tensor_scalar_mul(o_sb[:], ups[:, :O], r_sb[:, j : j + 1])
                nc.vector.tensor_add(o_sb[:], o_sb[:], bv_sb[:])
                nc.sync.dma_start(out[ds(P * qs, P), :], o_sb[:])

        def body(_iv=None):
            v_sb = vpool.tile([P, NKT, O], bf16, name="v_sb")
            st = {}
            for sb in range(NQB):
                proj(sb, v_sb, st)
                if sb >= 1:
                    finish(sb - 1, v_sb, st)
                scores(sb, st)
            finish(NQB - 1, v_sb, st)

        if reps == 1:
            body()
        else:
            with tc.For_i(0, reps, 1, hint_engines=(mybir.EngineType.PE,)) as iv:
                body(iv)

    nc.compile()
    return nc


def _get_nc(s=S, reps=1, loop_phase="all"):
    key = (s, reps, loop_phase)
    if key not in _CACHE:
        _CACHE[key] = _build_nc(s, reps, loop_phase)
    return _CACHE[key]


def make_mask(qblk=512):
    kp = np.arange(P)[:, None]
    qf = np.arange(qblk)[None, :]
    m = np.stack([(qf >= P * i + kp) for i in range(4)], axis=0)
    return m.astype(ml_dtypes.bfloat16)


def make_in_maps(x, Wq, bq, Wk, bk, Wv, bv, s=S):
    bf = ml_dtypes.bfloat16
    f8 = ml_dtypes.float8_e4m3
    x, Wq, bq, Wk, bk, Wv, bv = (
        np.asarray(a, dtype=np.float32) for a in (x, Wq, bq, Wk, bk, Wv, bv)
    )
    M = (Wq.astype(np.float64) @ Wk.T.astype(np.float64)).astype(np.float32)
    wc = (Wk @ bq).astype(np.float32)
    m8 = np.ascontiguousarray(np.clip(32.0 * M, -240, 240).astype(f8))
    wv_b = np.ascontiguousarray(Wv.astype(bf))
    wcp = np.ascontiguousarray((32.0 * wc).reshape(D // P, P).T.astype(np.float32))
    bv_rep = np.ascontiguousarray(np.broadcast_to(bv, (P, O)))
    mask = make_mask()
    in_maps = []
    for b in range(x.shape[0]):
        xT_b = np.ascontiguousarray(x[b].T.astype(bf))
        x8_b = np.ascontiguousarray(np.clip(x[b].T, -240, 240).astype(f8))
        in_maps.append(
            dict(xT=xT_b, x8=x8_b, m8=m8, wv=wv_b, wcp=wcp, bv_rep=bv_rep, mask=mask)
        )
    return in_maps


def kernel(x, Wq, bq, Wk, bk, Wv, bv):
    from concourse.bass_utils import run_bass_kernel_spmd

    x = np.asarray(x, dtype=np.float32)
    assert x.shape == (B, S, D), x.shape
    nc = _get_nc(S)
    in_maps = make_in_maps(x, Wq, bq, Wk, bk, Wv, bv)
    res = run_bass_kernel_spmd(nc, in_maps, core_ids=list(range(N_CORES)))
    return np.stack([res.results[c]["out"] for c in range(N_CORES)], axis=0)


# revision 6
# speedup vs baseline: 4.4399x; 4.4399x over previous
"""Causal scaled-dot-product attention on 8 NeuronCores (Trainium2, Bass/Tile).

Problem: x[8, 2048, 1024] f32, Wq/Wk[1024,1024], Wv[1024,512] (+biases).
  Q = xWq + bq; K = xWk + bk; V = xWv + bv
  out = softmax(causal(QK^T / sqrt(1024))) @ V          -> [8, 2048, 512] f32

Sharding: data-parallel over batch; core b handles batch element b.

Algebraic reduction (softmax is invariant to terms constant over k):
  QK^T = (xWq + bq)(xWk + bk)^T
       = x (Wq Wk^T) x^T  +  [x Wk bq]_k  +  (q-only terms, cancel in softmax)
so with M = Wq Wk^T and wc = Wk bq precomputed on the host:
  softmax_k(QK^T/32) = softmax_k( (x M x^T)/32 + c ),  c[k] = x[k]·wc / 32.
The c bias is folded into the A = x(32M) projection: adding the constant
row 32·wc[e] to every q-row of A^T makes the scores matmul produce
scores + 1024·c[k] directly (Σ_e x[k,e]·32wc[e] = 1024·c[k]), so c costs
one fused per-partition scalar-add on the PSUM->SBUF copy and nothing on
the PE.

Precision split (error budget: harness gate is rel_l2 < 2e-2):
  - The logit chain (A = x(32M) projection and scores = A x^T) runs in
    fp8e4m3 with MatmulPerfMode.DoubleRow (2 fp8 MACs/PE/cycle => 2x the
    bf16 matmul rate; each DoubleRow instruction contracts 2 k-subtiles
    of 128 via [128, 2, w] operand slices). fp8 noise on the logits only
    perturbs softmax weights (~1.5e-2 rel on the output; measured).
    M is pre-scaled by 32 on the host so its values sit in fp8e4m3's
    normal range; the exp activation scale (1/1024) compensates.
  - E = exp(scores) and V stay bf16: their relative error passes through
    to the output 1:1, so fp8 there would blow the 2e-2 budget. Hence the
    V projection and the U = E^T V matmul stay at the bf16 rate.

Per-core layout (all matmul contractions on the partition dim):
  - host supplies xT = x[b].T twice: bf16 (V-proj stationary) and fp8
    (A-proj moving, scores stationary); 32M fp8, Wv bf16, 32wc f32
  - scores^T tile [k=128, q<=512] = sum_h x8[:,2h:2h+2,kslice]^T @ a8
  - E^T = exp(scores^T/1024) (one ACT op: scale + cast to bf16); causal
    via 0/1 bf16 mask multiply on diagonal tiles (logits are O(1) so no
    running-max subtraction is needed)
  - Z: es[p,q] = sum_kt E^T tiles (DVE bf16 adds, off the PE), then ONE
    PE reduction ones[128,1]^T @ es per q-block; transposed to
    per-partition form with 4 tiny ap=1 matmuls into one PSUM bank
  - U[q-tile,128 x o,512] = sum_k E^T[k,q-tile]^T @ V[k-tile, o]  (PE)
  - out = U * (1/Z) + bv   (DVE per-partition scale + add)
Bias bv is folded at the end (softmax rows sum to 1 exactly).

Schedule: projection (sb), attention (qb) and normalization/output
(finish) phases are interleaved per 512-row block so the PE never waits
on the ACT/DVE tails:  proj(0) sc(0) proj(1) fin(0) sc(1) proj(2) fin(1)
sc(2) proj(3) fin(2) sc(3) fin(3).  Double-buffered tile pools let
consecutive For_i iterations overlap.

reps>1 wraps the body in a hardware For_i loop — used only by test.py to
measure per-iteration HW time free of dispatch overhead.
"""

import numpy as np
import ml_dtypes

B = 8
S = 2048
D = 1024
O = 512
P = 128
N_CORES = 8

_CACHE = {}


def _build_nc(s=S, reps=1, loop_phase="all"):
    from contextlib import ExitStack

    import concourse.tile as tile
    import concourse.mybir as mybir
    from concourse import bacc
    from concourse.bass import ds, ts

    assert loop_phase in ("all", "proj", "attn"), loop_phase

    f32 = mybir.dt.float32
    bf16 = mybir.dt.bfloat16
    f8 = mybir.dt.float8e4
    AF = mybir.ActivationFunctionType
    DR = mybir.MatmulPerfMode.DoubleRow

    DO = D // P            # 8 d-tiles
    EO = D // P            # 8 e-tiles
    HO = DO // 2           # 4 DoubleRow d-tile pairs
    QBLK = 512             # q-block width (moving free dim)
    NQB = s // QBLK        # q-blocks
    NKT = s // P           # k-tiles

    nc = bacc.Bacc(None, target_bir_lowering=False, debug=False)

    xT = nc.dram_tensor("xT", (D, s), bf16, kind="ExternalInput")
    x8_d = nc.dram_tensor("x8", (D, s), f8, kind="ExternalInput")
    m8_d = nc.dram_tensor("m8", (D, D), f8, kind="ExternalInput")
    wv = nc.dram_tensor("wv", (D, O), bf16, kind="ExternalInput")
    wcp_d = nc.dram_tensor("wcp", (P, EO), f32, kind="ExternalInput")
    bv_rep = nc.dram_tensor("bv_rep", (P, O), f32, kind="ExternalInput")
    mask = nc.dram_tensor("mask", (4, P, QBLK), bf16, kind="ExternalInput")
    out = nc.dram_tensor("out", (s, O), f32, kind="ExternalOutput")

    with tile.TileContext(nc) as tc, ExitStack() as ctx:
        persist = ctx.enter_context(tc.tile_pool(name="persist", bufs=1))
        apool = ctx.enter_context(tc.tile_pool(name="apool", bufs=2))
        vpool = ctx.enter_context(tc.tile_pool(name="vpool", bufs=2))
        etp = ctx.enter_context(tc.tile_pool(name="et", bufs=2))
        esp = ctx.enter_context(tc.tile_pool(name="esp", bufs=2))
        small = ctx.enter_context(tc.tile_pool(name="small", bufs=4))
        outp = ctx.enter_context(tc.tile_pool(name="outp", bufs=3))
        psAcc = ctx.enter_context(tc.tile_pool(name="psAcc", bufs=6, space="PSUM"))
        psZ = ctx.enter_context(tc.tile_pool(name="psZ", bufs=1, space="PSUM"))
        psT = ctx.enter_context(tc.tile_pool(name="psT", bufs=1, space="PSUM"))

        x8_sb = persist.tile([P, DO, s], f8)      # x^T fp8 (logit chain)
        m_sb = persist.tile([P, DO, D], f8)       # 32*(Wq Wk^T) fp8
        xT_sb = persist.tile([P, DO, s], bf16)    # x^T bf16 (V-proj)
        wv_sb = persist.tile([P, DO, O], bf16)
        wcp_sb = persist.tile([P, EO], f32)       # 32*(Wk bq), e-tile-major
        mask_sb = persist.tile([P, 4, QBLK], bf16)
        bv_sb = persist.tile([P, O], f32)
        ones_sb = persist.tile([P, 1], bf16)
        onef_sb = persist.tile([1, 1], f32)

        m_r = m8_d.rearrange("(do p) e -> p do e", p=P)
        x8_r = x8_d.rearrange("(do p) s -> p do s", p=P)
        xT_r = xT.rearrange("(do p) s -> p do s", p=P)
        wv_r = wv.rearrange("(do p) o -> p do o", p=P)
        # A-proj inputs first: compute can start as soon as m8+x8 land.
        for do in range(DO):
            nc.sync.dma_start(m_sb[:, do], m_r[:, do])
            nc.sync.dma_start(x8_sb[:, do], x8_r[:, do])
        nc.sync.dma_start(wcp_sb[:], wcp_d[:])
        for do in range(DO):
            nc.sync.dma_start(xT_sb[:, do], xT_r[:, do])
            nc.sync.dma_start(wv_sb[:, do], wv_r[:, do])
        nc.sync.dma_start(mask_sb[:], mask.rearrange("m p q -> p m q"))
        nc.sync.dma_start(bv_sb[:], bv_rep[:])
        nc.vector.memset(ones_sb[:], 1.0)
        nc.vector.memset(onef_sb[:], 1.0)

        def proj(sb, v_sb, st):
            ssl = ds(QBLK * sb, QBLK)
            a_t = apool.tile([P, EO, QBLK], f8, name="a_t")
            st["a", sb] = a_t
            for eo in range(EO):
                ps = psAcc.tile([P, QBLK], f32, tag="acc", name="ps_a")
                for h in range(HO):
                    nc.tensor.matmul(
                        ps[:], lhsT=m_sb[:, 2 * h : 2 * h + 2, ts(eo, P)],
                        rhs=x8_sb[:, 2 * h : 2 * h + 2, ssl],
                        start=(h == 0), stop=(h == HO - 1), perf_mode=DR,
                    )
                # fused c-bias: A'[e,q] = A[e,q] + 32*wc[e]
                nc.vector.tensor_scalar_add(
                    a_t[:, eo, :], ps[:], wcp_sb[:, eo : eo + 1]
                )
            for stt in range(QBLK // P):
                ps = psAcc.tile([P, QBLK], f32, tag="acc", name="ps_v")
                for do in range(DO):
                    nc.tensor.matmul(
                        ps[:, :O],
                        lhsT=xT_sb[:, do, ds(QBLK * sb + P * stt, P)],
                        rhs=wv_sb[:, do, :],
                        start=(do == 0), stop=(do == DO - 1),
                    )
                nc.vector.tensor_copy(v_sb[:, sb * (QBLK // P) + stt, :], ps[:, :O])

        def scores(qb, st):
            nkt = 4 * qb + 4
            et = etp.tile([P, nkt, QBLK], bf16, name="et")
            es = esp.tile([P, QBLK], bf16, name="es")
            st["et", qb] = et
            st["es", qb] = es
            a_t = st["a", qb]
            for kt in range(nkt):
                # diagonal k-tiles only cover q >= 128*m (rest is masked out
                # anyway); off-diagonal tiles cover the full q-block.
                m = kt - 4 * qb
                q0 = max(m, 0) * P
                qw = QBLK - q0
                ps = psAcc.tile([P, QBLK], f32, tag="acc", name="ps_s")
                for h in range(HO):
                    nc.tensor.matmul(
                        ps[:, :qw], lhsT=x8_sb[:, 2 * h : 2 * h + 2, ts(kt, P)],
                        rhs=a_t[:, 2 * h : 2 * h + 2, q0:],
                        start=(h == 0), stop=(h == HO - 1), perf_mode=DR,
                    )
                nc.scalar.activation(
                    out=et[:, kt, q0:], in_=ps[:, :qw], func=AF.Exp,
                    scale=1.0 / 1024.0,
                )
                if m >= 0:
                    nc.vector.tensor_mul(
                        et[:, kt, q0:], et[:, kt, q0:], mask_sb[:, m, q0:]
                    )
                if kt == 0:
                    nc.vector.tensor_copy(es[:], et[:, 0, :])
                else:
                    nc.vector.tensor_add(es[:, q0:], es[:, q0:], et[:, kt, q0:])

        def finish(qb, v_sb, st):
            et = st["et", qb]
            es = st["es", qb]
            zps = psZ.tile([1, QBLK], f32, tag="zrow", name="zps")
            nc.tensor.matmul(zps[:], lhsT=ones_sb[:], rhs=es[:], start=True, stop=True)
            z_sb = small.tile([1, QBLK], f32, name="z_sb")
            nc.vector.tensor_copy(z_sb[:], zps[:])
            ztp = psT.tile([P, 4], f32, tag="tp", name="ztp")
            for j in range(QBLK // P):
                nc.tensor.matmul(
                    ztp[:, j : j + 1], lhsT=z_sb[:, ts(j, P)], rhs=onef_sb[:],
                    start=True, stop=True, skip_group_check=True,
                )
            r_sb = small.tile([P, 4], f32, name="r_sb")
            nc.vector.reciprocal(r_sb[:], ztp[:])
            for j in range(QBLK // P):
                qs = qb * (QBLK // P) + j
                ups = psAcc.tile([P, QBLK], f32, tag="acc", name="ups")
                for kt in range(qs + 1):
                    nc.tensor.matmul(
                        ups[:, :O], lhsT=et[:, kt, ts(j, P)], rhs=v_sb[:, kt, :],
                        start=(kt == 0), stop=(kt == qs),
                    )
                o_sb = outp.tile([P, O], f32, name="o_sb")
                nc.vector.tensor_scalar_mul(o_sb[:], ups[:, :O], r_sb[:, j : j + 1])
                nc.vector.tensor_add(o_sb[:], o_sb[:], bv_sb[:])
                nc.sync.dma_start(out[ds(P * qs, P), :], o_sb[:])

        if loop_phase == "attn":
            attn_a = [persist.tile([P, EO, QBLK], f8, name="a_fix") for _ in range(NQB)]
            attn_v = persist.tile([P, NKT, O], bf16, name="v_fix")
            for t in attn_a:
                nc.vector.memset(t[:], 0.25)
            nc.vector.memset(attn_v[:], 0.25)

        def body(_iv=None):
            st = {}
            if loop_phase == "attn":
                v_sb = attn_v
                for sb in range(NQB):
                    st["a", sb] = attn_a[sb]
            else:
                v_sb = vpool.tile([P, NKT, O], bf16, name="v_sb")
            for sb in range(NQB):
                if loop_phase != "attn":
                    proj(sb, v_sb, st)
                if loop_phase != "proj":
                    if sb >= 1:
                        finish(sb - 1, v_sb, st)
                    scores(sb, st)
            if loop_phase != "proj":
                finish(NQB - 1, v_sb, st)

        if reps == 1:
            body()
        else:
            with tc.For_i(0, reps, 1, hint_engines=(mybir.EngineType.PE,)) as iv:
                body(iv)

    nc.compile()
    return nc


def _get_nc(s=S, reps=1, loop_phase="all"):
    key = (s, reps, loop_phase)
    if key not in _CACHE:
        _CACHE[key] = _build_nc(s, reps, loop_phase)
    return _CACHE[key]


def make_mask(qblk=512):
    kp = np.arange(P)[:, None]
    qf = np.arange(qblk)[None, :]
    m = np.stack([(qf >= P * i + kp) for i in range(4)], axis=0)
    return m.astype(ml_dtypes.bfloat16)


def make_in_maps(x, Wq, bq, Wk, bk, Wv, bv, s=S):
    bf = ml_dtypes.bfloat16
    f8 = ml_dtypes.float8_e4m3
    x, Wq, bq, Wk, bk, Wv, bv = (
        np.asarray(a, dtype=np.float32) for a in (x, Wq, bq, Wk, bk, Wv, bv)
    )
    M = (Wq.astype(np.float64) @ Wk.T.astype(np.float64)).astype(np.float32)
    wc = (Wk @ bq).astype(np.float32)
    m8 = np.ascontiguousarray(np.clip(32.0 * M, -240, 240).astype(f8))
    wv_b = np.ascontiguousarray(Wv.astype(bf))
    wcp = np.ascontiguousarray((32.0 * wc).reshape(D // P, P).T.astype(np.float32))
    bv_rep = np.ascontiguousarray(np.broadcast_to(bv, (P, O)))
    mask = make_mask()
    in_maps = []
    for b in range(x.shape[0]):
        xT_b = np.ascontiguousarray(x[b].T.astype(bf))
        x8_b = np.ascontiguousarray(np.clip(x[b].T, -240, 240).astype(f8))
        in_maps.append(
            dict(xT=xT_b, x8=x8_b, m8=m8, wv=wv_b, wcp=wcp, bv_rep=bv_rep, mask=mask)
        )
    return in_maps


def kernel(x, Wq, bq, Wk, bk, Wv, bv):
    from concourse.bass_utils import run_bass_kernel_spmd

    x = np.asarray(x, dtype=np.float32)
    assert x.shape == (B, S, D), x.shape
    nc = _get_nc(S)
    in_maps = make_in_maps(x, Wq, bq, Wk, bk, Wv, bv)
    res = run_bass_kernel_spmd(nc, in_maps, core_ids=list(range(N_CORES)))
    return np.stack([res.results[c]["out"] for c in range(N_CORES)], axis=0)


# revision 7
# speedup vs baseline: 4.5656x; 1.0283x over previous
"""Causal scaled-dot-product attention on 8 NeuronCores (Trainium2, Bass/Tile).

Problem: x[8, 2048, 1024] f32, Wq/Wk[1024,1024], Wv[1024,512] (+biases).
  Q = xWq + bq; K = xWk + bk; V = xWv + bv
  out = softmax(causal(QK^T / sqrt(1024))) @ V          -> [8, 2048, 512] f32

Sharding: data-parallel over batch; core b handles batch element b.

Algebraic reduction (softmax is invariant to terms constant over k):
  QK^T = (xWq + bq)(xWk + bk)^T
       = x (Wq Wk^T) x^T  +  [x Wk bq]_k  +  (q-only terms, cancel in softmax)
so with M = Wq Wk^T and wc = Wk bq precomputed on the host:
  softmax_k(QK^T/32) = softmax_k( (x M x^T)/32 + c ),  c[k] = x[k]·wc / 32.
The c bias is folded into the A = x(32M) projection: adding the constant
row 32·wc[e] to every q-row of A^T makes the scores matmul produce
scores + 1024·c[k] directly (Σ_e x[k,e]·32wc[e] = 1024·c[k]), so c costs
one fused per-partition scalar-add on the PSUM->SBUF copy and nothing on
the PE.

Precision split (error budget: harness gate is rel_l2 < 2e-2):
  - The logit chain (A = x(32M) projection and scores = A x^T) runs in
    fp8e4m3 with MatmulPerfMode.DoubleRow (2 fp8 MACs/PE/cycle => 2x the
    bf16 matmul rate; each DoubleRow instruction contracts 2 k-subtiles
    of 128 via [128, 2, w] operand slices). fp8 noise on the logits only
    perturbs softmax weights (~1.5e-2 rel on the output; measured).
    M is pre-scaled by 32 on the host so its values sit in fp8e4m3's
    normal range; the exp activation scale (1/1024) compensates.
  - E = exp(scores) and V stay bf16: their relative error passes through
    to the output 1:1, so fp8 there would blow the 2e-2 budget. Hence the
    V projection and the U = E^T V matmul stay at the bf16 rate.

Per-core layout (all matmul contractions on the partition dim):
  - host supplies xT = x[b].T twice: bf16 (V-proj stationary) and fp8
    (A-proj moving, scores stationary); 32M fp8, Wv bf16, 32wc f32
  - scores^T tile [k=128, q<=512] = sum_h x8[:,2h:2h+2,kslice]^T @ a8
  - E^T = exp(scores^T/1024) (one ACT op: scale + cast to bf16); causal
    via 0/1 bf16 mask multiply on diagonal tiles (logits are O(1) so no
    running-max subtraction is needed)
  - Z: es[p,q] = sum_kt E^T tiles (DVE bf16 adds, off the PE), then ONE
    PE reduction ones[128,1]^T @ es per q-block; transposed to
    per-partition form with 4 tiny ap=1 matmuls into one PSUM bank
  - U[q-tile,128 x o,512] = sum_k E^T[k,q-tile]^T @ V[k-tile, o]  (PE)
  - out = U * (1/Z) + bv   (DVE per-partition scale + add)
Bias bv is folded at the end (softmax rows sum to 1 exactly).

Schedule: projection (sb), attention (qb) and normalization/output
(finish) phases are interleaved per 512-row block so the PE never waits
on the ACT/DVE tails:  proj(0) sc(0) proj(1) fin(0) sc(1) proj(2) fin(1)
sc(2) proj(3) fin(2) sc(3) fin(3).  Double-buffered tile pools let
consecutive For_i iterations overlap.

reps>1 wraps the body in a hardware For_i loop — used only by test.py to
measure per-iteration HW time free of dispatch overhead.
"""

import numpy as np
import ml_dtypes

B = 8
S = 2048
D = 1024
O = 512
P = 128
N_CORES = 8

_CACHE = {}


def _build_nc(s=S, reps=1, loop_phase="all"):
    from contextlib import ExitStack

    import concourse.tile as tile
    import concourse.mybir as mybir
    from concourse import bacc
    from concourse.bass import ds, ts

    assert loop_phase in ("all", "proj", "attn"), loop_phase

    f32 = mybir.dt.float32
    bf16 = mybir.dt.bfloat16
    f8 = mybir.dt.float8e4
    AF = mybir.ActivationFunctionType
    DR = mybir.MatmulPerfMode.DoubleRow

    DO = D // P            # 8 d-tiles
    EO = D // P            # 8 e-tiles
    HO = DO // 2           # 4 DoubleRow d-tile pairs
    QBLK = 512             # q-block width (moving free dim)
    NQB = s // QBLK        # q-blocks
    NKT = s // P           # k-tiles

    nc = bacc.Bacc(None, target_bir_lowering=False, debug=False)

    xT = nc.dram_tensor("xT", (D, s), bf16, kind="ExternalInput")
    x8_d = nc.dram_tensor("x8", (D, s), f8, kind="ExternalInput")
    m8_d = nc.dram_tensor("m8", (D, D), f8, kind="ExternalInput")
    wv = nc.dram_tensor("wv", (D, O), bf16, kind="ExternalInput")
    wcp_d = nc.dram_tensor("wcp", (P, EO), f32, kind="ExternalInput")
    bv_rep = nc.dram_tensor("bv_rep", (P, O), f32, kind="ExternalInput")
    mask = nc.dram_tensor("mask", (4, P, QBLK), bf16, kind="ExternalInput")
    out = nc.dram_tensor("out", (s, O), f32, kind="ExternalOutput")

    with tile.TileContext(nc) as tc, ExitStack() as ctx:
        persist = ctx.enter_context(tc.tile_pool(name="persist", bufs=1))
        apool = ctx.enter_context(tc.tile_pool(name="apool", bufs=2))
        vpool = ctx.enter_context(tc.tile_pool(name="vpool", bufs=2))
        etp = ctx.enter_context(tc.tile_pool(name="et", bufs=2))
        esp = ctx.enter_context(tc.tile_pool(name="esp", bufs=2))
        small = ctx.enter_context(tc.tile_pool(name="small", bufs=4))
        outp = ctx.enter_context(tc.tile_pool(name="outp", bufs=3))
        psAcc = ctx.enter_context(tc.tile_pool(name="psAcc", bufs=6, space="PSUM"))
        psZ = ctx.enter_context(tc.tile_pool(name="psZ", bufs=1, space="PSUM"))
        psT = ctx.enter_context(tc.tile_pool(name="psT", bufs=1, space="PSUM"))

        x8_sb = persist.tile([P, DO, s], f8)      # x^T fp8 (logit chain)
        m_sb = persist.tile([P, DO, D], f8)       # 32*(Wq Wk^T) fp8
        xT_sb = persist.tile([P, DO, s], bf16)    # x^T bf16 (V-proj)
        wv_sb = persist.tile([P, DO, O], bf16)
        wcp_sb = persist.tile([P, EO], f32)       # 32*(Wk bq), e-tile-major
        mask_sb = persist.tile([P, 4, QBLK], bf16)
        bv_sb = persist.tile([P, O], f32)
        ones_sb = persist.tile([P, 1], bf16)
        onef_sb = persist.tile([1, 1], f32)

        m_r = m8_d.rearrange("(do p) e -> p do e", p=P)
        x8_r = x8_d.rearrange("(do p) s -> p do s", p=P)
        xT_r = xT.rearrange("(do p) s -> p do s", p=P)
        wv_r = wv.rearrange("(do p) o -> p do o", p=P)
        # A-proj inputs first: compute can start as soon as m8+x8 land.
        for do in range(DO):
            nc.sync.dma_start(m_sb[:, do], m_r[:, do])
            nc.sync.dma_start(x8_sb[:, do], x8_r[:, do])
        nc.sync.dma_start(wcp_sb[:], wcp_d[:])
        for do in range(DO):
            nc.sync.dma_start(xT_sb[:, do], xT_r[:, do])
            nc.sync.dma_start(wv_sb[:, do], wv_r[:, do])
        nc.sync.dma_start(mask_sb[:], mask.rearrange("m p q -> p m q"))
        nc.sync.dma_start(bv_sb[:], bv_rep[:])
        nc.vector.memset(ones_sb[:], 1.0)
        nc.vector.memset(onef_sb[:], 1.0)

        def proj(sb, v_sb, st):
            ssl = ds(QBLK * sb, QBLK)
            a_t = apool.tile([P, EO, QBLK], f8, name="a_t")
            st["a", sb] = a_t
            for eo in range(EO):
                ps = psAcc.tile([P, QBLK], f32, tag="acc", name="ps_a")
                for h in range(HO):
                    nc.tensor.matmul(
                        ps[:], lhsT=m_sb[:, 2 * h : 2 * h + 2, ts(eo, P)],
                        rhs=x8_sb[:, 2 * h : 2 * h + 2, ssl],
                        start=(h == 0), stop=(h == HO - 1), perf_mode=DR,
                    )
                # fused c-bias: A'[e,q] = A[e,q] + 32*wc[e]
                nc.vector.tensor_scalar_add(
                    a_t[:, eo, :], ps[:], wcp_sb[:, eo : eo + 1]
                )
            for stt in range(QBLK // P):
                ps = psAcc.tile([P, QBLK], f32, tag="acc", name="ps_v")
                for do in range(DO):
                    nc.tensor.matmul(
                        ps[:, :O],
                        lhsT=xT_sb[:, do, ds(QBLK * sb + P * stt, P)],
                        rhs=wv_sb[:, do, :],
                        start=(do == 0), stop=(do == DO - 1),
                    )
                nc.vector.tensor_copy(v_sb[:, sb * (QBLK // P) + stt, :], ps[:, :O])

        def scores(qb, st):
            nkt = 4 * qb + 4
            et = etp.tile([P, nkt, QBLK], bf16, name="et")
            es = esp.tile([P, QBLK], bf16, name="es")
            st["et", qb] = et
            st["es", qb] = es
            a_t = st["a", qb]
            for kt in range(nkt):
                # diagonal k-tiles only cover q >= 128*m (rest is masked out
                # anyway); off-diagonal tiles cover the full q-block.
                m = kt - 4 * qb
                q0 = max(m, 0) * P
                qw = QBLK - q0
                ps = psAcc.tile([P, QBLK], f32, tag="acc", name="ps_s")
                for h in range(HO):
                    nc.tensor.matmul(
                        ps[:, :qw], lhsT=x8_sb[:, 2 * h : 2 * h + 2, ts(kt, P)],
                        rhs=a_t[:, 2 * h : 2 * h + 2, q0:],
                        start=(h == 0), stop=(h == HO - 1), perf_mode=DR,
                    )
                nc.scalar.activation(
                    out=et[:, kt, q0:], in_=ps[:, :qw], func=AF.Exp,
                    scale=1.0 / 1024.0,
                )
                if m >= 0:
                    nc.vector.tensor_mul(
                        et[:, kt, q0:], et[:, kt, q0:], mask_sb[:, m, q0:]
                    )
                if kt == 0:
                    nc.vector.tensor_copy(es[:], et[:, 0, :])
                else:
                    nc.vector.tensor_add(es[:, q0:], es[:, q0:], et[:, kt, q0:])

        def finish(qb, v_sb, st):
            et = st["et", qb]
            es = st["es", qb]
            # Z row first, then the j=0 U chain runs on the PE while DVE
            # copies Z out of PSUM -- the tiny transpose matmuls never wait.
            zps = psZ.tile([1, QBLK], f32, tag="zrow", name="zps")
            nc.tensor.matmul(zps[:], lhsT=ones_sb[:], rhs=es[:], start=True, stop=True)
            z_sb = small.tile([1, QBLK], f32, name="z_sb")
            nc.vector.tensor_copy(z_sb[:], zps[:])

            ups0 = psAcc.tile([P, QBLK], f32, tag="acc", name="ups")
            qs0 = qb * (QBLK // P)
            for kt in range(qs0 + 1):
                nc.tensor.matmul(
                    ups0[:, :O], lhsT=et[:, kt, ts(0, P)], rhs=v_sb[:, kt, :],
                    start=(kt == 0), stop=(kt == qs0),
                )

            ztp = psT.tile([P, 4], f32, tag="tp", name="ztp")
            for j in range(QBLK // P):
                nc.tensor.matmul(
                    ztp[:, j : j + 1], lhsT=z_sb[:, ts(j, P)], rhs=onef_sb[:],
                    start=True, stop=True, skip_group_check=True,
                )
            r_sb = small.tile([P, 4], f32, name="r_sb")
            nc.vector.reciprocal(r_sb[:], ztp[:])

            o_sb = outp.tile([P, O], f32, name="o_sb")
            nc.vector.tensor_scalar_mul(o_sb[:], ups0[:, :O], r_sb[:, 0:1])
            nc.vector.tensor_add(o_sb[:], o_sb[:], bv_sb[:])
            nc.sync.dma_start(out[ds(P * qs0, P), :], o_sb[:])

            for j in range(1, QBLK // P):
                qs = qb * (QBLK // P) + j
                ups = psAcc.tile([P, QBLK], f32, tag="acc", name="ups")
                for kt in range(qs + 1):
                    nc.tensor.matmul(
                        ups[:, :O], lhsT=et[:, kt, ts(j, P)], rhs=v_sb[:, kt, :],
                        start=(kt == 0), stop=(kt == qs),
                    )
                o_sb = outp.tile([P, O], f32, name="o_sb")
                nc.vector.tensor_scalar_mul(o_sb[:], ups[:, :O], r_sb[:, j : j + 1])
                nc.vector.tensor_add(o_sb[:], o_sb[:], bv_sb[:])
                nc.sync.dma_start(out[ds(P * qs, P), :], o_sb[:])

        if loop_phase == "attn":
            attn_a = [persist.tile([P, EO, QBLK], f8, name="a_fix") for _ in range(NQB)]
            attn_v = persist.tile([P, NKT, O], bf16, name="v_fix")
            for t in attn_a:
                nc.vector.memset(t[:], 0.25)
            nc.vector.memset(attn_v[:], 0.25)

        def body(_iv=None):
            st = {}
            if loop_phase == "attn":
                v_sb = attn_v
                for sb in range(NQB):
                    st["a", sb] = attn_a[sb]
            else:
                v_sb = vpool.tile([P, NKT, O], bf16, name="v_sb")
            for sb in range(NQB):
                if loop_phase != "attn":
                    proj(sb, v_sb, st)
                if loop_phase != "proj":
                    if sb >= 1:
                        finish(sb - 1, v_sb, st)
                    scores(sb, st)
            if loop_phase != "proj":
                finish(NQB - 1, v_sb, st)

        if reps == 1:
            body()
        else:
            with tc.For_i(0, reps, 1, hint_engines=(mybir.EngineType.PE,)) as iv:
                body(iv)

    nc.compile()
    return nc


def _get_nc(s=S, reps=1, loop_phase="all"):
    key = (s, reps, loop_phase)
    if key not in _CACHE:
        _CACHE[key] = _build_nc(s, reps, loop_phase)
    return _CACHE[key]


def make_mask(qblk=512):
    kp = np.arange(P)[:, None]
    qf = np.arange(qblk)[None, :]
    m = np.stack([(qf >= P * i + kp) for i in range(4)], axis=0)
    return m.astype(ml_dtypes.bfloat16)


def make_in_maps(x, Wq, bq, Wk, bk, Wv, bv, s=S):
    bf = ml_dtypes.bfloat16
    f8 = ml_dtypes.float8_e4m3
    x, Wq, bq, Wk, bk, Wv, bv = (
        np.asarray(a, dtype=np.float32) for a in (x, Wq, bq, Wk, bk, Wv, bv)
    )
    M = (Wq.astype(np.float64) @ Wk.T.astype(np.float64)).astype(np.float32)
    wc = (Wk @ bq).astype(np.float32)
    m8 = np.ascontiguousarray(np.clip(32.0 * M, -240, 240).astype(f8))
    wv_b = np.ascontiguousarray(Wv.astype(bf))
    wcp = np.ascontiguousarray((32.0 * wc).reshape(D // P, P).T.astype(np.float32))
    bv_rep = np.ascontiguousarray(np.broadcast_to(bv, (P, O)))
    mask = make_mask()
    in_maps = []
    for b in range(x.shape[0]):
        xT_b = np.ascontiguousarray(x[b].T.astype(bf))
        x8_b = np.ascontiguousarray(np.clip(x[b].T, -240, 240).astype(f8))
        in_maps.append(
            dict(xT=xT_b, x8=x8_b, m8=m8, wv=wv_b, wcp=wcp, bv_rep=bv_rep, mask=mask)
        )
    return in_maps


def kernel(x, Wq, bq, Wk, bk, Wv, bv):
    from concourse.bass_utils import run_bass_kernel_spmd

    x = np.asarray(x, dtype=np.float32)
    assert x.shape == (B, S, D), x.shape
    nc = _get_nc(S)
    in_maps = make_in_maps(x, Wq, bq, Wk, bk, Wv, bv)
    res = run_bass_kernel_spmd(nc, in_maps, core_ids=list(range(N_CORES)))
    return np.stack([res.results[c]["out"] for c in range(N_CORES)], axis=0)
